# revision 20
# baseline (speedup 1.0000x reference)
"""Trainium2 Bass kernel for nn_Policy_11484742550172.

The reference pads each input channel with 100 zeros on the right and keeps
the last 32 columns — with 100 >= 32 the conv input is exactly zero for any
x, so the network collapses to a weights-only dense chain:

    v1 = relu(conv1_b)                                  [8]
    v2 = relu(sum_k conv2_w[:, :, k] @ v1 + conv2_b)    [16]
    v3 = relu(sum_k conv3_w[:, :, k] @ v2 + conv3_b)    [32]
    v4 = relu(conv4_w[:, :, 0] @ v3 + conv4_b)          [32]
    h   = relu(fc1_w.reshape(128, 32, 30).sum(-1) @ v4 + fc1_b)
    out = softmax(fc2_w @ h + fc2_b)
        = sigmoid([l0 - l1, l1 - l0])   (softmax over 2 = sigmoid of diff)

This is an exact algebraic simplification (conv of zeros = bias), not an
approximation. x and conv1_w never influence the output.

Schedule notes:
- Small weights/biases are host-packed into one [128, 137] tensor (one DMA);
  fc1_w (99% of the bytes) ships unmodified, split over the SWDGE and
  SP-HWDGE rings (the ACT ring is left free for the ACT table load).
- Conv chain runs on PE + ScalarE while DVE does the fc1 group-sum
  reductions chunk-by-chunk as the DMAs land, then 32x32 block transposes.
- relu/sigmoid live in one ACT table set, warmed during the DMA window.

Sharding: the problem is far too small to shard; the kernel is replicated
SPMD on all 8 cores and core 0's output is returned.
"""

import numpy as np

import concourse.bass as bass
import concourse.tile as tile
from concourse import bacc, mybir
from concourse.bass_utils import run_bass_kernel_spmd

N_CORES = 8
F32 = mybir.dt.float32
ALU = mybir.AluOpType
ACT = mybir.ActivationFunctionType
X = mybir.AxisListType.X

_CACHE = {}


def _build():
    nc = bacc.Bacc(
        "TRN2",
        target_bir_lowering=False,
        debug=False,
        num_devices=N_CORES,
        enable_partition_id=False,
        num_swdge_queues=2,
    )

    pkd = nc.dram_tensor("pk", [128, 137], F32, kind="ExternalInput")
    fw1d = nc.dram_tensor("fc1_w", [128, 960], F32, kind="ExternalInput")
    outd = nc.dram_tensor("out", [1, 2], F32, kind="ExternalOutput")

    with tile.TileContext(nc) as tc:
        with (
            tc.tile_pool(name="sb", bufs=1) as sb,
            tc.tile_pool(name="ps", bufs=1, space="PSUM") as ps,
        ):
            zero = nc.const_aps.aps[(F32, 0.0)]
            one = nc.const_aps.aps[(F32, 1.0)]

            # Warm the sigmoid_and_others ACT table (covers relu/sigmoid)
            # while the DMAs are in flight.
            warm = sb.tile([1, 1], F32)
            nc.scalar.activation(warm[:], zero[:1, :1], ACT.Sigmoid)

            # --- loads: pack first on SWDGE, fc1_w split over both rings ---
            pk = sb.tile([128, 137], F32)
            nc.gpsimd.dma_start(pk[:], pkd[:])
            fw1 = sb.tile([128, 960], F32)
            nc.sync.dma_start(fw1[:, 480:720], fw1d[:, 480:720])
            nc.gpsimd.dma_start(fw1[:, 0:240], fw1d[:, 0:240])
            nc.sync.dma_start(fw1[:, 720:960], fw1d[:, 720:960])
            nc.gpsimd.dma_start(fw1[:, 240:480], fw1d[:, 240:480])

            fc1b = pk[:, 0:1]
            b1 = pk[0:8, 1:2]
            b2 = pk[0:16, 2:3]
            b3 = pk[0:32, 3:4]
            b4 = pk[0:32, 4:5]
            fw2t = pk[:, 5:7]
            fb2r = pk[0:1, 7:9]
            w2v = pk[0:8, 9:41].rearrange("i (o k) -> i o k", k=2)
            w3v = pk[0:16, 41:105].rearrange("i (o k) -> i o k", k=2)
            w4t = pk[0:32, 105:137]

            # --- conv chain on PE + ScalarE ---
            v1 = sb.tile([8, 1], F32)
            nc.scalar.activation(v1[:], b1, ACT.Relu)

            w2s = sb.tile([8, 16], F32)
            nc.vector.tensor_reduce(out=w2s[:], in_=w2v, axis=X, op=ALU.add)
            p2 = ps.tile([16, 1], F32)
            nc.tensor.matmul(p2[:], w2s[:], v1[:], start=True, stop=True)
            v2 = sb.tile([16, 1], F32)
            nc.scalar.activation(v2[:], p2[:], ACT.Relu, bias=b2)

            w3s = sb.tile([16, 32], F32)
            nc.vector.tensor_reduce(out=w3s[:], in_=w3v, axis=X, op=ALU.add)
            p3 = ps.tile([32, 1], F32)
            nc.tensor.matmul(p3[:], w3s[:], v2[:], start=True, stop=True)
            v3 = sb.tile([32, 1], F32)
            nc.scalar.activation(v3[:], p3[:], ACT.Relu, bias=b3)

            p4 = ps.tile([32, 1], F32)
            nc.tensor.matmul(p4[:], w4t, v3[:], start=True, stop=True)
            v4 = sb.tile([32, 1], F32)
            nc.scalar.activation(v4[:], p4[:], ACT.Relu, bias=b4)

            # --- fc2 logit-difference prep (early, on DVE) ---
            dwp = sb.tile([128, 2], F32)
            nc.vector.tensor_tensor(
                out=dwp[:, 0:1], in0=fw2t[:, 0:1], in1=fw2t[:, 1:2], op=ALU.subtract
            )
            nc.vector.tensor_tensor(
                out=dwp[:, 1:2], in0=fw2t[:, 1:2], in1=fw2t[:, 0:1], op=ALU.subtract
            )
            dbp = sb.tile([1, 2], F32)
            nc.vector.tensor_tensor(
                out=dbp[:, 0:1], in0=fb2r[:, 0:1], in1=fb2r[:, 1:2], op=ALU.subtract
            )
            nc.vector.tensor_tensor(
                out=dbp[:, 1:2], in0=fb2r[:, 1:2], in1=fb2r[:, 0:1], op=ALU.subtract
            )

            # --- fc1: group-sum fc1_w over the 30 repeated positions (DVE,
            # chunked to follow the DMAs), then 32x32 block transposes ---
            w1r = sb.tile([128, 32], F32)
            fw1v = fw1[:].rearrange("p (o t) -> p o t", t=30)
            for lo, hi in ((16, 24), (0, 8), (24, 32), (8, 16)):
                nc.vector.tensor_reduce(
                    out=w1r[:, lo:hi], in_=fw1v[:, lo:hi], axis=X, op=ALU.add
                )

            w1t = sb.tile([32, 128], F32)
            for c in range(4):
                nc.vector.transpose(
                    w1t[:, c * 32 : (c + 1) * 32], w1r[c * 32 : (c + 1) * 32, :]
                )

            py = ps.tile([128, 1], F32)
            nc.tensor.matmul(py[:], w1t[:], v4[:], start=True, stop=True)
            h = sb.tile([128, 1], F32)
            nc.scalar.activation(h[:], py[:], ACT.Relu, bias=fc1b)

            # --- fc2 logit difference + softmax(2) == sigmoid ---
            pl = ps.tile([1, 2], F32)
            nc.tensor.matmul(pl[:], h[:], dwp[:], start=True, stop=False)
            nc.tensor.matmul(pl[:], one[:1, :1], dbp[:], start=False, stop=True)

            probs = sb.tile([1, 2], F32)
            nc.scalar.activation(probs[:], pl[:], ACT.Sigmoid)
            nc.scalar.dma_start(outd[:], probs[:])

    nc.compile()
    return nc


def _in_map(inputs):
    def f(name):
        return np.asarray(inputs[name], dtype=np.float32)

    pk = np.zeros((128, 137), dtype=np.float32)
    pk[:, 0] = f("fc1_b")
    pk[0:8, 1] = f("conv1_b")
    pk[0:16, 2] = f("conv2_b")
    pk[0:32, 3] = f("conv3_b")
    pk[0:32, 4] = f("conv4_b")
    pk[:, 5:7] = f("fc2_w").T
    pk[0, 7:9] = f("fc2_b")
    pk[0:8, 9:41] = f("conv2_w").transpose(1, 0, 2).reshape(8, 32)
    pk[0:16, 41:105] = f("conv3_w").transpose(1, 0, 2).reshape(16, 64)
    pk[0:32, 105:137] = f("conv4_w").reshape(32, 32).T

    return {
        "pk": pk,
        "fc1_w": np.ascontiguousarray(f("fc1_w")),
    }


def kernel(**inputs) -> np.ndarray:
    if "nc" not in _CACHE:
        _CACHE["nc"] = _build()
    nc = _CACHE["nc"]
    in_map = _in_map(inputs)
    res = run_bass_kernel_spmd(
        nc,
        [dict(in_map) for _ in range(N_CORES)],
        core_ids=list(range(N_CORES)),
    )
    return res.results[0]["out"].reshape(2).astype(np.float32)
